# revision 85
# baseline (speedup 1.0000x reference)
"""Causal multi-head attention (RoPE) forward for Trainium2, 8 NeuronCores.

Problem: B=2, T=2048, C=1024, H=16, D=64.  out = proj(softmax(rope(q) rope(k)^T / 8, causal) @ v)

Sharding: 8 cores = 2 batches x 4 head-groups (4 heads each); qkv
column-sharded per head group, proj row-sharded; the host sums the 4
per-group partial projections per batch.

Design (vs the bf16 baseline, 174067ns -> 128308ns):
 - qkv projections run as fp8e4m3 DoubleRow matmuls (0.5 cyc/row, verified
   on hw) with a residual split: q,k ~= xh@(wh*2^8) + xl@wh (2-term,
   ~1.3% rel err), v = 3-term incl. xh@wl (~0.1%). The power-of-2 weight
   rescale makes all terms share one PSUM accumulation at product scale
   2^12; the 1/4096 is folded into the rope tables (q,k) and the v
   PSUM->SBUF copy. DoubleRow k-tile-pair packing is done on the host for
   both x and w (w additionally ft-major so partial loads stay contiguous).
 - Attention per (head-pair, 512-query chunk): the two heads' S^T tiles
   [128 keys, 512 q] occupy the two banks of one [128,2,512] span tile
   (each QK+mask group confined to its own bank), so a single ACT
   instruction exps both heads (88 exps total); the causal mask is folded
   into the QK accumulation group as one PE matmul that adds -2^16 over the
   shared upper-triangle constant, so exp emits exact zeros and no
   elementwise mask op exists. AV uses P^T tiles as lhsT (y[q,d]
   layout, 65-wide streams incl. a ones column for the softmax denominator),
   halving AV PE time vs the y^T layout.
 - PSUM rule learned the hard way: hardware start_tensor_calc zeroes the
   whole 2KB bank, so each AV accumulation group owns one rotating bank
   tile; pool-slot WAR plus same-region WAW keeps concurrent groups in
   distinct banks.
 - Normalization: per-region [128,1] reciprocal + per-partition
   tensor_scalar; y is PE-transposed (identity rhs) back to [c,t] for the
   output projection, emitted per query-subtile for tail overlap.
 - Scheduling: the next chunk's qkv/v units are interleaved as fillers into
   the attention span loop; chunk 0 borrows the idle span pool for a 3-deep
   qkv pipeline during the DMA-bound start; DMA order minimizes
   time-to-first-QK; a memset-fed warmup ramps the PE p-state from ~0.3us.
 - gpsimd cannot touch PSUM (walrus crashes): it only carries the rope
   cos-mul off a one-time PSUM->SBUF bounce; DVE carries the PSUM-side
   elementwise work; tail copies ride the by-then idle ACT engine.
"""

import numpy as np
import ml_dtypes

_CACHE = {}

B, T, C = 2, 2048, 1024
HLOC, D = 4, 64            # heads per core, head dim
GC = HLOC * D              # 256 channels per group
P = 128
NTT = T // P               # 16 token tiles
NIC = T // 512             # 4 query chunks of 512
THETA = 10000.0
N_CORES = 8
F8SCALE = 4096.0           # shared product scale of the 3 residual terms
QK3 = False                # q/k residual terms: True = 3-term (~0.1% err),
                           # False = 2-term hh+lh (~1.3% err, -6.8us PE)


def _rope_tables():
    freqs = 1.0 / THETA ** (np.arange(0, D, 2, dtype=np.float32) / D)
    t = np.arange(T, dtype=np.float32)
    f = np.outer(t, freqs)                          # [T, 32]
    emb = np.concatenate([f, f], axis=-1)           # [T, 64]
    cosT = np.cos(emb).T.astype(np.float32)         # [64, T]
    sinT = np.sin(emb).T.astype(np.float32)
    # tile to 128 partitions (2 heads per partition block)
    return (np.concatenate([cosT, cosT], 0), np.concatenate([sinT, sinT], 0))





def _build_program():
    import concourse.bass as bass
    import concourse.mybir as mybir
    import concourse.tile as tile

    dt = mybir.dt
    fp32 = dt.float32
    bf16 = dt.bfloat16
    f8 = dt.float8e4
    EXP = mybir.ActivationFunctionType.Exp
    MUL = mybir.AluOpType.mult
    DR = mybir.MatmulPerfMode.DoubleRow

    nc = bass.Bass("TRN2", target_bir_lowering=False, debug=False,
                   enable_asserts=True, num_devices=N_CORES)

    xh_d = nc.dram_tensor("xh", [P, 8, T], f8, kind="ExternalInput").ap()
    xl_d = nc.dram_tensor("xl", [P, 8, T], f8, kind="ExternalInput").ap()
    # q/k weights ft-major so per-ft loads are contiguous 1KB/partition runs
    wqA_d = nc.dram_tensor("wqA", [P, 4, 8, P], f8, kind="ExternalInput").ap()
    wql_d = nc.dram_tensor("wql", [P, 4, 8, P], f8, kind="ExternalInput").ap()
    wqB_d = nc.dram_tensor("wqB", [P, 4, 8, P], f8, kind="ExternalInput").ap()
    wvA_d = nc.dram_tensor("wvA", [P, 8, 256], f8, kind="ExternalInput").ap()
    wvl_d = nc.dram_tensor("wvl", [P, 8, 256], f8, kind="ExternalInput").ap()
    wvB_d = nc.dram_tensor("wvB", [P, 8, 256], f8, kind="ExternalInput").ap()
    rmat_d = nc.dram_tensor("rmat", [P, P], bf16, kind="ExternalInput").ap()
    ident_d = nc.dram_tensor("ident", [P, P], bf16, kind="ExternalInput").ap()
    wpT_d = nc.dram_tensor("wpT", [GC, C], bf16, kind="ExternalInput").ap()
    cosT_d = nc.dram_tensor("cosT", [P, T], bf16, kind="ExternalInput").ap()
    sinT_d = nc.dram_tensor("sinT", [P, T], bf16, kind="ExternalInput").ap()
    mask_d = nc.dram_tensor("mask", [P, P], bf16, kind="ExternalInput").ap()
    out_d = nc.dram_tensor("out", [T, C], fp32, kind="ExternalOutput").ap()
    import os
    dbg = os.environ.get("KERNEL_DEBUG_DUMPS") == "1"
    if dbg:
        dqk_d = [nc.dram_tensor(f"dqk{i}", [P, T], bf16, kind="ExternalOutput").ap()
                 for i in range(4)]
        dv_d = nc.dram_tensor("dv", [P, NTT, 4, D + 1], bf16, kind="ExternalOutput").ap()
        dyn_d = nc.dram_tensor("dyn", [P, NIC, 4, GC], bf16, kind="ExternalOutput").ap()

    # PSUM discipline: hardware start_tensor_calc ZEROES the whole 2KB bank
    # region, so every accumulation group owns a full bank tile from start to
    # stop. Pool-slot rotation (WAR on reuse) + same-region WAW between group
    # members is what keeps concurrent groups in distinct banks.
    with tile.TileContext(nc) as tc:
        with (
            tc.tile_pool(name="persist", bufs=1) as persist,
            tc.tile_pool(name="work", bufs=8) as work,
            tc.tile_pool(name="ptp", bufs=33) as ptp,
            tc.tile_pool(name="outp", bufs=4) as outp,
            tc.tile_pool(name="qp", bufs=2, space="PSUM") as qp,
            tc.tile_pool(name="sp", bufs=2, space="PSUM") as spp,
            tc.tile_pool(name="avp", bufs=2, space="PSUM") as avp,
        ):
            # ---- persistent SBUF loads (in first-use order) ----------------
            # p-state warmup on a memset tile: starts ~0.3us in, no DMA dep
            wz = work.tile([P, P], bf16, tag="wz", name="wz")
            nc.vector.memset(wz[:], 0.5)
            warm = qp.tile([P, P], fp32, tag="mm", name="warmup")
            for i in range(40):
                nc.tensor.matmul(warm[:], wz[:], wz[:],
                                 start=True, stop=True, skip_group_check=True)
            rmat_sb = persist.tile([P, P], bf16, tag="rmat")
            nc.sync.dma_start(rmat_sb[:], rmat_d[:])
            # load order minimizes time-to-first-QK on the serialized DMA
            # device: ft01 weights, x0-hi, ft01 lo-weights, x0-lo, ft23
            # weights, rope tables, then the rest in first-use order.
            wqA_sb = persist.tile([P, 4, 8, P], f8, tag="wqA")
            wql_sb = persist.tile([P, 4, 8, P], f8, tag="wql") if QK3 else None
            wqB_sb = persist.tile([P, 4, 8, P], f8, tag="wqB")
            xh_sb = persist.tile([P, 8, T], f8, tag="xh")
            xl_sb = persist.tile([P, 8, T], f8, tag="xl")
            cos_sb = persist.tile([P, T], bf16, tag="cos")
            sin_sb = persist.tile([P, T], bf16, tag="sin")
            nc.sync.dma_start(wqA_sb[:, 0:2], wqA_d[:, 0:2])
            nc.sync.dma_start(xh_sb[:, :, 0:512], xh_d[:, :, 0:512])
            if QK3:
                nc.sync.dma_start(wql_sb[:, 0:2], wql_d[:, 0:2])
            nc.sync.dma_start(wqB_sb[:, 0:2], wqB_d[:, 0:2])
            nc.sync.dma_start(xl_sb[:, :, 0:512], xl_d[:, :, 0:512])
            nc.sync.dma_start(wqA_sb[:, 2:4], wqA_d[:, 2:4])
            if QK3:
                nc.sync.dma_start(wql_sb[:, 2:4], wql_d[:, 2:4])
            nc.sync.dma_start(wqB_sb[:, 2:4], wqB_d[:, 2:4])
            nc.sync.dma_start(cos_sb[:, 0:512], cosT_d[:, 0:512])
            nc.sync.dma_start(sin_sb[:, 0:512], sinT_d[:, 0:512])
            nc.sync.dma_start(cos_sb[:, 512:T], cosT_d[:, 512:T])
            nc.sync.dma_start(sin_sb[:, 512:T], sinT_d[:, 512:T])
            mask_sb = persist.tile([P, P], bf16, tag="mask")
            nc.sync.dma_start(mask_sb[:], mask_d[:])
            ident_sb = persist.tile([P, P], bf16, tag="ident")
            nc.sync.dma_start(ident_sb[:], ident_d[:])
            wvA_sb = persist.tile([P, 8, 256], f8, tag="wvA")
            nc.sync.dma_start(wvA_sb[:], wvA_d[:])
            wvl_sb = persist.tile([P, 8, 256], f8, tag="wvl")
            nc.sync.dma_start(wvl_sb[:], wvl_d[:])
            wvB_sb = persist.tile([P, 8, 256], f8, tag="wvB")
            nc.sync.dma_start(wvB_sb[:], wvB_d[:])
            wpT_sb = persist.tile([P, 2, C], bf16, tag="wpT")
            nc.sync.dma_start(wpT_sb[:], wpT_d.rearrange("(cb p) o -> p cb o", p=P))
            for tcix in range(1, NIC):
                cs = slice(512 * tcix, 512 * (tcix + 1))
                nc.sync.dma_start(xh_sb[:, :, cs], xh_d[:, :, cs])
                nc.sync.dma_start(xl_sb[:, :, cs], xl_d[:, :, cs])
            ones_sb = persist.tile([P, 1], bf16, tag="ones")
            nc.vector.memset(ones_sb[:], 1.0)

            # rope outputs: q^T,k^T per 2-head block [128, T] bf16
            qk_rope = [persist.tile([P, T], bf16, tag=f"qkrope{i}", name=f"qkrope{i}")
                       for i in range(4)]
            if dbg:
                dyn_sb = persist.tile([P, NIC, 4, GC], bf16, tag="dyn")
            # v with a ones column per head: [128 part=t, 16 ttiles, 4, 65];
            # the ones column makes the softmax denominator column 64 of each
            # AV accumulation group.
            v_sb = persist.tile([P, NTT, 4, D + 1], bf16, tag="vsb")
            nc.vector.memset(v_sb[:, :, :, D:D + 1], 1.0)

            if QK3:
                QKTERMS = ((wqA_sb, xh_sb), (wql_sb, xh_sb), (wqB_sb, xl_sb))
            else:
                QKTERMS = ((wqA_sb, xh_sb), (wqB_sb, xl_sb))
            VTERMS = ((xh_sb, wvA_sb), (xh_sb, wvl_sb), (xl_sb, wvB_sb))
            NQT = len(QKTERMS)

            def emit_qkv_ft(tcix, ft):
                """q/k projection + rope for one 128-row block (ft 0,1 q; 2,3 k)."""
                cs = slice(512 * tcix, 512 * (tcix + 1))
                # chunk 0 runs before any attention: borrow the idle span
                # pool for a 3-deep qkv pipeline during the DMA-bound start
                pool = spp if tcix == 0 else qp
                tg = "sp" if tcix == 0 else "mm"
                psq = pool.tile([P, 512], fp32, tag=tg, name=f"psq_{ft}_{tcix}")
                for ti, (wsb, xsb) in enumerate(QKTERMS):
                    for g in range(4):
                        nc.tensor.matmul(
                            psq[:], wsb[:, ft, 2 * g:2 * g + 2, :],
                            xsb[:, 2 * g:2 * g + 2, cs], perf_mode=DR,
                            start=(ti == 0 and g == 0),
                            stop=(ti == NQT - 1 and g == 3))
                # one PSUM->SBUF bounce so the cos table-mul runs on the
                # otherwise idle gpsimd engine (it cannot read PSUM); the
                # sin-mul stays on DVE to keep the rotate chain short
                qsb = work.tile([P, 512], bf16, tag="qsb")
                nc.vector.tensor_copy(out=qsb[:], in_=psq[:])
                t1 = work.tile([P, 512], bf16, tag="t1")
                nc.gpsimd.tensor_tensor(t1[:], qsb[:], cos_sb[:, cs], MUL)
                u = work.tile([P, 512], bf16, tag="u")
                nc.vector.tensor_tensor(u[:], psq[:], sin_sb[:, cs], MUL)
                # rotate matmul reuses psq in place (qsb/u reads done)
                nc.tensor.matmul(psq[:], rmat_sb[:], u[:], start=True, stop=True)
                nc.vector.tensor_add(qk_rope[ft][:, cs], psq[:], t1[:])

            def emit_v_tt(tt):
                psv = qp.tile([P, 256], fp32, tag="mm", name=f"psv_{tt}")
                for ti, (lsb, rsb) in enumerate(VTERMS):
                    for g in range(4):
                        nc.tensor.matmul(
                            psv[:], lsb[:, 2 * g:2 * g + 2, P * tt:P * (tt + 1)],
                            rsb[:, 2 * g:2 * g + 2, :], perf_mode=DR,
                            start=(ti == 0 and g == 0), stop=(ti == 2 and g == 3))
                nc.vector.tensor_scalar(
                    out=v_sb[:, tt, :, 0:D],
                    in0=psv[:].rearrange("p (h d) -> p h d", d=D),
                    scalar1=1.0 / F8SCALE, scalar2=None, op0=MUL)

            def emit_qkv(tcix):
                for ft in range(4):
                    emit_qkv_ft(tcix, ft)
                for tt in range(4 * tcix, 4 * tcix + 4):
                    emit_v_tt(tt)

            COPY = mybir.ActivationFunctionType.Copy

            def emit_transproj(ic, qs, ynorm):
                """transpose + project + store one 128-query subtile. For the
                last chunk the PSUM->SBUF copies ride the by-then idle ACT
                engine to shorten the tail chain."""
                tail = ic == NIC - 1
                trT = qp.tile([P, 2, P], bf16, tag="mm", name=f"tr_{ic}_{qs}")
                yT = work.tile([P, 2, P], bf16, tag="yT", name=f"yT_{ic}_{qs}")
                for cb in range(2):
                    nc.tensor.transpose(trT[:, cb, :], ynorm[:, P * cb:P * (cb + 1)],
                                        ident_sb[:])
                    nc.vector.tensor_copy(out=yT[:, cb, :], in_=trT[:, cb, :])
                rows = slice(512 * ic + P * qs, 512 * ic + P * (qs + 1))
                for oc in range(2):
                    ppool, ptag = (qp, "mm") if tail else (avp, "av")
                    pp = ppool.tile([P, 512], fp32, tag=ptag, name=f"pp_{ic}_{qs}_{oc}")
                    for cb in range(2):
                        nc.tensor.matmul(
                            pp[:], yT[:, cb, :],
                            wpT_sb[:, cb, 512 * oc:512 * (oc + 1)],
                            start=(cb == 0), stop=(cb == 1))
                    ob = outp.tile([P, 512], fp32, tag="ob", name=f"ob_{ic}_{qs}_{oc}")
                    if tail:
                        nc.scalar.activation(ob[:], pp[:], COPY)
                    else:
                        nc.vector.tensor_copy(out=ob[:], in_=pp[:])
                    nc.sync.dma_start(out_d[rows, 512 * oc:512 * (oc + 1)], ob[:])

            def emit_attention(ic, fillers=()):
                """Per head-pair: QK+mask groups write the two banks of one
                [128,2,512] span tile (each group confined to its bank), so a
                single ACT instruction exps both heads' S^T; then one AV
                accumulation group per (head, query subtile), each owning a
                rotating bank tile with a ones column for the denominator."""
                njb = 4 * ic + 4
                fillers = list(fillers)
                ynorms = [work.tile([P, GC], bf16, tag="ynorm",
                                    name=f"yn_{ic}_{qs}") for qs in range(4)]
                pts_all = []
                for hp in range(2):
                    qrow = qk_rope[hp]
                    krow = qk_rope[2 + hp]
                    pts = []
                    for jt in range(njb):
                        m = jt - 4 * ic
                        qo, qw = ((128, 384) if m == 1 else (256, 256) if m == 2
                                  else ((384, 128) if m == 3 else (0, 512)))
                        sp = spp.tile([P, 2, 512], fp32, tag="sp",
                                      name=f"sp_{hp}_{ic}_{jt}")
                        for u in range(2):
                            hb = 64 * u
                            nc.tensor.matmul(
                                sp[:, u, qo:qo + qw],
                                krow[hb:hb + 64, P * jt:P * (jt + 1)],
                                qrow[hb:hb + 64, 512 * ic + qo:512 * ic + qo + qw],
                                start=True, stop=(m < 0))
                            if m >= 0:
                                nc.tensor.matmul(
                                    sp[:, u, 128 * m:128 * (m + 1)],
                                    ident_sb[:], mask_sb[:],
                                    start=False, stop=True)
                        pt = ptp.tile([P, 2, 512], bf16, tag="pt",
                                      name=f"pt_{hp}_{ic}_{jt}")
                        nc.scalar.activation(pt[:, :, qo:qo + qw],
                                             sp[:, :, qo:qo + qw],
                                             EXP, scale=0.125)
                        pts.append(pt)
                        if fillers and (ic > 0 and hp == 0 or ic == 0 and hp == 1):
                            fillers.pop(0)()
                    pts_all.append(pts)
                # leftover fillers (ic0: chunk-1 inputs land only ~20us in)
                # drain BEFORE the AV blocks so the next chunk's qkv starts
                # while the AV/norm stream chews this chunk
                for f in fillers:
                    f()
                fillers = []
                # AV groups emitted after BOTH pairs' QK/exp streams so the
                # in-order PE stream never parks exp-paced AV members in
                # front of ready QKs
                for hp in range(2):
                    for u in range(2):
                        h = 2 * hp + u
                        for qs in range(4):
                            jmax = 4 * ic + qs
                            av = avp.tile([P, 512], fp32, tag="av",
                                          name=f"av_{ic}_{qs}_{h}")
                            for jt in range(jmax + 1):
                                nc.tensor.matmul(
                                    av[:, 0:D + 1],
                                    pts_all[hp][jt][:, u, 128 * qs:128 * (qs + 1)],
                                    v_sb[:, jt, h, :],
                                    start=(jt == 0), stop=(jt == jmax))
                            recip = work.tile([P, 1], fp32, tag="recip",
                                              name=f"rc_{ic}_{qs}_{h}")
                            nc.vector.reciprocal(recip[:], av[:, D:D + 1])
                            nc.vector.tensor_scalar(
                                out=ynorms[qs][:, 64 * h:64 * (h + 1)],
                                in0=av[:, 0:D], scalar1=recip[:],
                                scalar2=None, op0=MUL)
                            if dbg and h == 3:
                                nc.vector.tensor_copy(out=dyn_sb[:, ic, qs, :],
                                                      in_=ynorms[qs][:])
                            if h == 3:
                                emit_transproj(ic, qs, ynorms[qs])

            # chunk c+1's qkv/v units are interleaved INTO attention(c)'s
            # span loop as fillers, so qp pool slots rotate in
            # execution-overlap order and QKs are never starved behind a
            # monolithic qkv stream.
            emit_qkv(0)
            for ic in range(NIC):
                fillers = []
                if ic + 1 < NIC:
                    nxt = ic + 1
                    fts = [(lambda f=ft, t=nxt: emit_qkv_ft(t, f)) for ft in range(4)]
                    vts = [(lambda t=tt: emit_v_tt(t)) for tt in range(4 * nxt, 4 * nxt + 4)]
                    # v units between ft pairs so the qp rotation isn't
                    # gated by the previous ft's rope reads
                    fillers = [fts[0], fts[1], vts[0], vts[1],
                               fts[2], fts[3], vts[2], vts[3]]
                emit_attention(ic, fillers)
            if dbg:
                for i in range(4):
                    nc.sync.dma_start(dqk_d[i][:], qk_rope[i][:])
                nc.sync.dma_start(dv_d[:], v_sb[:])
                nc.sync.dma_start(dyn_d[:], dyn_sb[:])

    _split_excess_waits(nc)
    return nc


def _split_excess_waits(nc, maxw=1):
    """Walrus codegen rejects instructions carrying >1 sem wait; move excess
    waits onto no-ops inserted immediately before, on the same engine."""
    import concourse.mybir as mybir
    n = 0
    for f in nc.m.functions:
        for bb in f.blocks:
            new = []
            for inst in bb.instructions:
                si = getattr(inst, "sync_info", None)
                if si is not None and si.on_wait and len(si.on_wait) > maxw:
                    waits = list(si.on_wait)
                    excess, keep = waits[:-maxw], waits[-maxw:]
                    for i in range(0, len(excess), maxw):
                        new.append(mybir.InstNoOp(
                            name=f"{inst.name}_wsp{n}_{i}", engine=inst.engine,
                            bass_nofuse=True,
                            sync_info=mybir.SyncInfo(on_wait=excess[i:i + maxw],
                                                     on_update=[])))
                    si.on_wait = keep
                    n += 1
                new.append(inst)
            bb.instructions[:] = new
    return n


def _get_runner():
    """Build the Bass program once and wrap it in a shard_map-jitted callable
    over the 8 cores (mirrors concourse.bass2jax.run_bass_via_pjrt)."""
    if "runner" in _CACHE:
        return _CACHE["runner"]
    import jax
    import numpy as _np
    from jax.sharding import Mesh, PartitionSpec
    from jax.experimental.shard_map import shard_map
    import concourse.mybir as mybir
    from concourse.bass2jax import _bass_exec_p, install_neuronx_cc_hook

    install_neuronx_cc_hook()
    from concourse.bass2jax import partition_id_tensor
    nc = _build_program()

    part_name = nc.partition_id_tensor.name if nc.partition_id_tensor else None
    in_names, out_names, out_avals = [], [], []
    for alloc in nc.m.functions[0].allocations:
        if not isinstance(alloc, mybir.MemoryLocationSet):
            continue
        name = alloc.memorylocations[0].name
        if alloc.kind == "ExternalInput":
            if name != part_name:
                in_names.append(name)
        elif alloc.kind == "ExternalOutput":
            out_names.append(name)
            out_avals.append(jax.core.ShapedArray(
                tuple(alloc.tensor_shape), mybir.dt.np(alloc.dtype)))
    n_params = len(in_names)
    all_names = in_names + out_names
    if part_name is not None:
        all_names = all_names + [part_name]

    def _body(*args):
        operands = list(args)
        if part_name is not None:
            operands.append(partition_id_tensor())
        outs = _bass_exec_p.bind(
            *operands, out_avals=tuple(out_avals), in_names=tuple(all_names),
            out_names=tuple(out_names), lowering_input_output_aliases=(),
            sim_require_finite=True, sim_require_nnan=True, nc=nc)
        return tuple(outs)

    devices = jax.devices()[:N_CORES]
    mesh = Mesh(_np.asarray(devices), ("core",))
    n_outs = len(out_names)
    sharded = jax.jit(
        shard_map(_body, mesh=mesh,
                  in_specs=(PartitionSpec("core"),) * (n_params + n_outs),
                  out_specs=(PartitionSpec("core"),) * n_outs,
                  check_rep=False),
        donate_argnums=tuple(range(n_params, n_params + n_outs)),
        keep_unused=True)

    runner = (sharded, in_names, out_names, out_avals)
    _CACHE["runner"] = runner
    return runner


def _pack_dr(a):
    """[C, M] -> DoubleRow-packed [128, C//128, M] (k-tile kt = row c//128)."""
    Cd, M = a.shape
    return np.ascontiguousarray(a.reshape(Cd // P, P, M).transpose(1, 0, 2))


def _pack_qk(a):
    """[C, 512] -> ft-major DoubleRow-packed [128, 4, C//128, 128]."""
    Cd, M = a.shape
    b = a.reshape(Cd // P, P, 4, P)                 # (kt, p, ft, m)
    return np.ascontiguousarray(b.transpose(1, 2, 0, 3))


def _q8(a, scale):
    f8 = ml_dtypes.float8_e4m3
    return (a * scale).astype(f8)


def _prepare_core_inputs(x, w_qkv, w_proj):
    bf = ml_dtypes.bfloat16
    f8 = ml_dtypes.float8_e4m3
    cosT, sinT = _CACHE.setdefault("rope", _rope_tables())
    # q_rope = q*cos + R(q * sinP), with sinP a half-swapped sin table and
    # R the block-diagonal rotate-half matrix applied by one PE matmul.
    sinP = np.concatenate([sinT[D // 2:D], sinT[:D // 2]], axis=0)
    sinP = np.concatenate([sinP, sinP], axis=0)[:P]
    # fold the fp8 residual product scale into the rope tables
    cosc = (cosT / F8SCALE).astype(bf)
    sinc = (sinP / F8SCALE).astype(bf)
    R = np.zeros((D, D), np.float32)
    for d in range(D // 2):
        R[d, d + D // 2] = -1.0
        R[d + D // 2, d] = 1.0
    R_pair = np.zeros((P, P), np.float32)
    R_pair[:D, :D] = R
    R_pair[D:, D:] = R
    rmat = np.ascontiguousarray(R_pair.T).astype(bf)
    ident = np.eye(P, dtype=np.float32).astype(bf)
    mask = np.where(np.arange(P)[None, :] >= np.arange(P)[:, None],
                    0.0, -65536.0).astype(np.float32).astype(bf)

    # x residual split (shared across cores per batch)
    xs = []
    for b in range(B):
        xT = np.ascontiguousarray(x[b].T).astype(np.float32)      # [C, T]
        xh8 = _q8(xT, 1.0)
        xl8 = _q8((xT - xh8.astype(np.float32)) * 256.0, 1.0)
        xs.append((_pack_dr(xh8), _pack_dr(xl8)))

    per_core = []
    for core in range(N_CORES):
        b, g = divmod(core, 4)
        rows = slice(GC * g, GC * (g + 1))
        wq = w_qkv[0 * C:1 * C][rows].astype(np.float32)
        wk = w_qkv[1 * C:2 * C][rows].astype(np.float32)
        wv = w_qkv[2 * C:3 * C][rows].astype(np.float32)

        def wsplit(w, ftmajor):              # w: [out, C] -> wT [C, out]
            wT = np.ascontiguousarray(w.T)
            wh8 = _q8(wT, 16.0)
            whA = (wh8.astype(np.float32) * 256.0).astype(f8)     # exact *2^8
            wl8 = _q8(wT - wh8.astype(np.float32) / 16.0, F8SCALE)
            pack = _pack_qk if ftmajor else _pack_dr
            return (pack(whA), pack(wl8), pack(wh8))

        wqk = np.concatenate([wq, wk], axis=0)                    # [512, C]
        wqkA, wqkl, wqkB = wsplit(wqk, True)
        wvA, wvl, wvB = wsplit(wv, False)
        wpT = np.ascontiguousarray(w_proj[:, rows].T).astype(bf)  # [256, C]
        xh_p, xl_p = xs[b]
        per_core.append({
            "xh": xh_p, "xl": xl_p,
            "wqA": wqkA, "wql": wqkl, "wqB": wqkB,
            "wvA": wvA, "wvl": wvl, "wvB": wvB,
            "rmat": rmat, "ident": ident, "wpT": wpT,
            "cosT": cosc, "sinT": sinc, "mask": mask})
    return per_core


def _run_cores(per_core):
    from concourse import bass_utils
    if "nc" not in _CACHE:
        from concourse.bass2jax import install_neuronx_cc_hook
        install_neuronx_cc_hook()
        _CACHE["nc"] = _build_program()
    res = bass_utils.run_bass_kernel_spmd(
        _CACHE["nc"], per_core, core_ids=list(range(N_CORES)))
    return res.results


def kernel(x, w_qkv, w_proj):
    x = np.asarray(x, dtype=np.float32)
    w_qkv = np.asarray(w_qkv, dtype=np.float32)
    w_proj = np.asarray(w_proj, dtype=np.float32)
    per_core = _prepare_core_inputs(x, w_qkv, w_proj)
    results = _run_cores(per_core)
    out = np.zeros((B, T, C), dtype=np.float32)
    for core in range(N_CORES):
        b = core // 4
        out[b] += results[core]["out"]
    return out
